# revision 1
# baseline (speedup 1.0000x reference)
"""KNN classification kernel for Trainium2 (8 NeuronCores).

Problem: B=1024 queries x N=200000 gallery, D=256, top-10 neighbors,
softmax-weighted one-hot class scores over 50 classes.

Math fold: reference computes gallery = l2norm(train.T, axis=1) -- i.e. each
feature dim d is normalized by ||train[:, d]|| over the FULL gallery. That
scale folds into the query side:
    sim[b, n] = sum_d (q[b,d]/||q[b]||) * train[n,d] / ||train[:,d]||
              = q_scaled[b] . train[n]
so the device kernel is a pure matmul + top-k screen.

Device (per core, gallery sharded along N into 8 x 25000, zero-padded to
25088 = 49 x 512):
  PE: sim tile [128q, 512n] = q_scaled_bf16.T @ gallery_bf16 (2 K=128 steps),
      two tiles packed into one 2-bank PSUM slot [128, 1024]
  DVE: top-8 values per 1024-col region (InstMax) -> cand [1024, 25*8]
Host: screen top-J candidate values -> identify regions -> recompute those
  regions' sims exactly in f64 -> exact top-10 -> softmax -> class scores.
Safety: a true top-10 item's region has region-max >= item value, so the
region ranks <=10 among all regions by top value -- top-J>=16 region
screening provably covers the true top-10 (modulo bf16 noise, which is
~40 sigma below the rank-10/16 value gaps; verified empirically).
"""

import os
import numpy as np

NB_KNN = 10
T = 0.07
NUM_CLASSES = 50
EPS = 1e-12

B, N, D = 1024, 200000, 256
NCORES = 8
NPC = N // NCORES          # 25000 real cols per core
TILE = 512
NPC_PAD = 25088            # 49 * 512
NT = NPC_PAD // TILE       # 49 tiles per core
BLOCKS = [8, 8, 8, 8, 8, 8, 1]   # tiles per DMA block
NREG = 13                  # 12 grouped regions (2048 cols) + 1 single (512)
TOPJ = 16                  # regions screened per query
GROUP = 4                  # psum tiles per DVE max8 region

_CACHE = {}


def _build_bass():
    import concourse.bacc as bacc
    import concourse.tile as tile
    from concourse import mybir

    nc = bacc.Bacc("TRN2")
    bf16 = mybir.dt.bfloat16
    f32 = mybir.dt.float32

    g_d = nc.dram_tensor("g", [2, 128, NPC_PAD], bf16, kind="ExternalInput")
    q_d = nc.dram_tensor("q", [2, 128, B], bf16, kind="ExternalInput")
    cand_d = nc.dram_tensor("cand", [B, NREG * 8], f32, kind="ExternalOutput")

    with tile.TileContext(nc) as tc:
        with tc.tile_pool(name="qp", bufs=1) as qp, \
             tc.tile_pool(name="gp", bufs=2) as gp, \
             tc.tile_pool(name="cp", bufs=8) as cp, \
             tc.tile_pool(name="pp", bufs=1, space="PSUM") as pp:
            q0 = qp.tile([128, B], bf16, tag="q0")
            q1 = qp.tile([128, B], bf16, tag="q1")
            nc.sync.dma_start(out=q0[:], in_=q_d[0])
            nc.sync.dma_start(out=q1[:], in_=q_d[1])

            cands = [cp.tile([128, NREG * 8], f32, tag="cand",
                             name=f"cand{i}") for i in range(8)]

            tbase = 0
            for blk, ntile in enumerate(BLOCKS):
                cw = ntile * TILE
                c0 = tbase * TILE
                g0 = gp.tile([128, cw], bf16, tag=f"g0_{ntile}")
                g1 = gp.tile([128, cw], bf16, tag=f"g1_{ntile}")
                nc.sync.dma_start(out=g0[:], in_=g_d[0][:, c0:c0 + cw])
                nc.sync.dma_start(out=g1[:], in_=g_d[1][:, c0:c0 + cw])
                for bc in range(8):
                    lhs0 = q0[:, bc * 128:(bc + 1) * 128]
                    lhs1 = q1[:, bc * 128:(bc + 1) * 128]
                    for p in range(0, ntile, GROUP):
                        grp = min(GROUP, ntile - p)
                        ps = pp.tile([128, TILE * GROUP], f32,
                                     tag="ps4", bufs=2)
                        for s in range(grp):
                            rsl = slice((p + s) * TILE, (p + s + 1) * TILE)
                            osl = slice(s * TILE, (s + 1) * TILE)
                            nc.tensor.matmul(ps[:, osl], lhs0, g0[:, rsl],
                                             start=True, stop=False)
                            nc.tensor.matmul(ps[:, osl], lhs1, g1[:, rsl],
                                             start=False, stop=True)
                        reg = (tbase + p) // GROUP
                        nc.vector.max(cands[bc][:, reg * 8:(reg + 1) * 8],
                                      ps[:, :TILE * grp])
                tbase += ntile

            for bc in range(8):
                nc.sync.dma_start(
                    out=cand_d[bc * 128:(bc + 1) * 128, :], in_=cands[bc][:])
    if not nc.is_finalized():
        nc.finalize()
    return nc


def _run_device(g_shards, q_packed):
    from concourse.bass_utils import run_bass_kernel_spmd
    if "nc" not in _CACHE:
        _CACHE["nc"] = _build_bass()
    nc = _CACHE["nc"]
    in_maps = [{"g": g_shards[c], "q": q_packed} for c in range(NCORES)]
    res = run_bass_kernel_spmd(nc, in_maps, list(range(NCORES)))
    return np.concatenate(
        [res.results[c]["cand"] for c in range(NCORES)], axis=1)


def _run_emulated(g_shards, q_packed):
    qf = q_packed.astype(np.float32).reshape(256, B)
    out = []
    for c in range(NCORES):
        gf = g_shards[c].astype(np.float32).reshape(256, NPC_PAD)
        sim = qf.T @ gf                                   # [B, NPC_PAD]
        res = np.empty((B, NREG * 8), np.float32)
        for r in range(NREG):
            a = r * 2048
            b = min(a + 2048, NPC_PAD)
            blkv = sim[:, a:b]
            top8 = -np.sort(-blkv, axis=1)[:, :8]
            res[:, r * 8:(r + 1) * 8] = top8
        out.append(res)
    return np.concatenate(out, axis=1)


def kernel(test_features, train_features, train_labels):
    test_features = np.asarray(test_features, dtype=np.float32)
    train_features = np.asarray(train_features, dtype=np.float32)
    train_labels = np.asarray(train_labels)

    import ml_dtypes
    bf16 = ml_dtypes.bfloat16

    # ---- host pre: fold normalizations into the query side ----
    tf64 = train_features.astype(np.float64)
    norm_d = np.maximum(np.sqrt(np.sum(tf64 * tf64, axis=0)), EPS)
    q64 = test_features.astype(np.float64)
    qn = np.sqrt(np.sum(q64 * q64, axis=1, keepdims=True))
    q_scaled = q64 / np.maximum(qn, EPS) / norm_d          # [B, D] f64

    q_packed = np.ascontiguousarray(
        q_scaled.T.astype(bf16).reshape(2, 128, B))
    gt = train_features.T.astype(bf16)                     # [D, N]
    g_shards = []
    for c in range(NCORES):
        sl = np.zeros((256, NPC_PAD), dtype=bf16)
        sl[:, :NPC] = gt[:, c * NPC:(c + 1) * NPC]
        g_shards.append(np.ascontiguousarray(sl.reshape(2, 128, NPC_PAD)))

    # ---- device: bf16 matmul + per-region top-8 screen ----
    if os.environ.get("KNN_EMULATE"):
        cand = _run_emulated(g_shards, q_packed)
    else:
        cand = _run_device(g_shards, q_packed)
    cand = cand.astype(np.float32)                         # [B, 8*NREG*8]

    # ---- host post: screen -> exact f64 rerank -> softmax scores ----
    topj = np.argpartition(-cand, TOPJ - 1, axis=1)[:, :TOPJ]
    reg_id = topj // 8                                     # 0..199 global

    reg_queries = {}
    for b in range(B):
        for r in set(reg_id[b].tolist()):
            reg_queries.setdefault(r, []).append(b)

    per_q_vals = [[] for _ in range(B)]
    per_q_cols = [[] for _ in range(B)]
    for r, qs in reg_queries.items():
        core, rc = divmod(r, NREG)
        c0 = core * NPC + rc * 2048
        c1 = core * NPC + min(rc * 2048 + 2048, NPC)
        block = tf64[c0:c1]                                # [w, D] view
        sims = q_scaled[qs] @ block.T                      # [nq, w] f64
        cols = np.arange(c0, c1)
        for i, b in enumerate(qs):
            per_q_vals[b].append(sims[i])
            per_q_cols[b].append(cols)

    labels = train_labels.astype(np.int64)
    scores = np.zeros((B, NUM_CLASSES), dtype=np.float64)
    for b in range(B):
        v = np.concatenate(per_q_vals[b])
        cidx = np.concatenate(per_q_cols[b])
        sel = np.argpartition(-v, NB_KNN - 1)[:NB_KNN]
        order = np.lexsort((cidx[sel], -v[sel]))
        sel = sel[order]
        topv = v[sel]
        w = np.exp(topv / T - np.max(topv) / T)
        w /= w.sum()
        np.add.at(scores[b], labels[cidx[sel]], w)
    return scores.astype(np.float32)


if __name__ == "__main__":
    rng = np.random.default_rng(0)
    tf = rng.standard_normal((B, D), dtype=np.float32)
    trf = rng.standard_normal((N, D), dtype=np.float32)
    trl = rng.integers(0, NUM_CLASSES, N).astype(np.int64)
    os.environ["KNN_EMULATE"] = "1"
    out = kernel(tf, trf, trl)
    print(out.shape, out.dtype, out.sum())



# revision 2
# speedup vs baseline: 1.8063x; 1.8063x over previous
"""KNN classification kernel for Trainium2 (8 NeuronCores), v2.

Problem: B=1024 queries x N=200000 gallery, D=256, top-10 neighbors,
softmax-weighted one-hot class scores over 50 classes.

Math fold: reference computes gallery = l2norm(train.T, axis=1) -- each
feature dim d is scaled by 1/||train[:, d]|| over the FULL gallery. That
folds into the query side, so the device only needs q_scaled @ train.T.

Device (per core, gallery sharded along N into 8 x 25000, zero-padded to
25600 = 25 granules x 1024):
  PE:  fp8e4 DoubleRow matmuls (K=256 packed as [128,2]) -> sim granule
       [128q, 1024] f32 in PSUM (0.5 cycles/row = 4x bf16 throughput)
  Screen (the bottleneck) split across two engines, alternating granules:
   - DVE tensor_reduce(max, axis=X) over [128,2,512] -> two per-512 maxes
   - ACT Relu(x - tau_b) with accum_out -> per-1024 exceedance sum, where
     tau_b = 3.25 * sigma_dev(b) is a per-query threshold (bias AP)
  PSUM ring: 4 granule buffers x 2 banks; PE runs ~2x faster than the
  consumers so the screen engines stay saturated.
Host: flag 512-blocks (DVE max >= tau) and 1024-granules (relu sum > 0),
  rescore flagged columns exactly in f64, exact top-10 -> softmax scores.
  Certificate: found 10th value must clear tau + 5.7 sigma_noise, else that
  query falls back to a full exact rescore (probability ~1e-7).
Safety: any exact-top-10 item has z >= ~3.9 sigma whp while tau = 3.25
  sigma; fp8 dot-product noise is ~0.06 sigma, so a top-10 item landing
  under the device-side threshold needs a ~10-sigma-noise deviation.
"""

import os
import numpy as np

NB_KNN = 10
T = 0.07
NUM_CLASSES = 50
EPS = 1e-12

B, N, D = 1024, 200000, 256
NCORES = 8
NPC = N // NCORES           # 25000 real cols per core
GR = 1024                   # granule width (2 PSUM banks)
NGR = 25                    # granules per chunk
NPC_PAD = GR * NGR          # 25600
NCH = 8                     # query chunks of 128
GBLK = 5 * GR               # gallery DMA block = 5 granules
TAU_Z = 3.25                # screen threshold in device-sigma units
CERT_Z = 0.33               # certificate margin in device-sigma units

_CACHE = {}


def _build_bass(double_row=True):
    import concourse.bacc as bacc
    import concourse.tile as tile
    from concourse import mybir

    nc = bacc.Bacc("TRN2")
    f32 = mybir.dt.float32
    fp8 = mybir.dt.float8e4

    g_d = nc.dram_tensor("g", [128, 2, NPC_PAD], fp8, kind="ExternalInput")
    q_d = nc.dram_tensor("q", [128, 2, B], fp8, kind="ExternalInput")
    tau_d = nc.dram_tensor("tau", [128, NCH], f32, kind="ExternalInput")
    dve_d = nc.dram_tensor("dve", [NCH, 128, 26], f32, kind="ExternalOutput")
    act_d = nc.dram_tensor("act", [NCH, 128, 13], f32, kind="ExternalOutput")

    X = mybir.AxisListType.X
    MAX = mybir.AluOpType.max
    RELU = mybir.ActivationFunctionType.Relu
    pm = mybir.MatmulPerfMode.DoubleRow if double_row else None

    with tile.TileContext(nc) as tc:
        with tc.tile_pool(name="qp", bufs=1) as qp, \
             tc.tile_pool(name="gp", bufs=1) as gp, \
             tc.tile_pool(name="op", bufs=1) as op, \
             tc.tile_pool(name="pp", bufs=1, space="PSUM") as pp:
            q8 = qp.tile([128, 2, B], fp8, tag="q8")
            tau = qp.tile([128, NCH], f32, tag="tau")
            nc.sync.dma_start(out=q8[:], in_=q_d[:])
            nc.sync.dma_start(out=tau[:], in_=tau_d[:])

            gal = []
            for i in range(NPC_PAD // GBLK):
                t = gp.tile([128, 2, GBLK], fp8, tag=f"gal{i}")
                nc.sync.dma_start(
                    out=t[:], in_=g_d[:, :, i * GBLK:(i + 1) * GBLK])
                gal.append(t)

            for c in range(NCH):
                dve_o = op.tile([128, 26], f32, tag=f"dve{c}")
                act_o = op.tile([128, 13], f32, tag=f"act{c}")
                lhs = q8[:, :, c * 128:(c + 1) * 128]
                for j in range(NGR):
                    blk = gal[j // 5]
                    l0 = (j % 5) * GR
                    ps = pp.tile([128, GR], f32, tag="ps", bufs=4)
                    if double_row:
                        nc.tensor.matmul(ps[:], lhs, blk[:, :, l0:l0 + GR],
                                         start=True, stop=True, perf_mode=pm)
                    else:
                        nc.tensor.matmul(ps[:], lhs[:, 0], blk[:, 0, l0:l0 + GR],
                                         start=True, stop=False)
                        nc.tensor.matmul(ps[:], lhs[:, 1], blk[:, 1, l0:l0 + GR],
                                         start=False, stop=True)
                    o = j // 2
                    if (j + c) % 2 == 0:
                        nc.vector.tensor_reduce(
                            dve_o[:, 2 * o:2 * o + 2],
                            ps[:].rearrange("p (r w) -> p r w", r=2),
                            axis=X, op=MAX)
                    else:
                        nc.scalar.activation(
                            out=ps[:], in_=ps[:], func=RELU,
                            bias=tau[:, c:c + 1], scale=1.0,
                            accum_out=act_o[:, o:o + 1])
                nc.sync.dma_start(out=dve_d[c], in_=dve_o[:])
                nc.sync.dma_start(out=act_d[c], in_=act_o[:])
    if not nc.is_finalized():
        nc.finalize()
    return nc


def _run_device(g_shards, q_packed, tau_packed):
    from concourse.bass_utils import run_bass_kernel_spmd
    if "nc" not in _CACHE:
        _CACHE["nc"] = _build_bass()
    nc = _CACHE["nc"]
    in_maps = [{"g": g_shards[c], "q": q_packed, "tau": tau_packed}
               for c in range(NCORES)]
    res = run_bass_kernel_spmd(nc, in_maps, list(range(NCORES)))
    return ([res.results[c]["dve"] for c in range(NCORES)],
            [res.results[c]["act"] for c in range(NCORES)])


def _run_emulated(g_shards, q_packed, tau_packed):
    """Numpy emulation of the device kernel (same outputs)."""
    qf = q_packed.astype(np.float32)         # [128, 2, B]
    dves, acts = [], []
    for core in range(NCORES):
        gf = g_shards[core].astype(np.float32)   # [128, 2, NPC_PAD]
        sim = np.einsum("pib,pin->bn", qf, gf)   # [B, NPC_PAD]
        dve = np.zeros((NCH, 128, 26), np.float32)
        act = np.zeros((NCH, 128, 13), np.float32)
        for c in range(NCH):
            sc = sim[c * 128:(c + 1) * 128]      # [128, NPC_PAD]
            for j in range(NGR):
                o = j // 2
                gsl = sc[:, j * GR:(j + 1) * GR]
                if (j + c) % 2 == 0:
                    m = gsl.reshape(128, 2, 512).max(axis=2)
                    dve[c, :, 2 * o:2 * o + 2] = m
                else:
                    bias = tau_packed[:, c:c + 1]
                    act[c, :, o] = np.maximum(gsl + bias, 0).sum(axis=1)
        dves.append(dve)
        acts.append(act)
    return dves, acts


def kernel(test_features, train_features, train_labels):
    import ml_dtypes
    FP8 = ml_dtypes.float8_e4m3fn

    test_features = np.asarray(test_features, dtype=np.float32)
    train_features = np.asarray(train_features, dtype=np.float32)
    labels = np.asarray(train_labels).astype(np.int64)

    # ---- host pre: fold normalizations into the query side ----
    tf64 = train_features.astype(np.float64)             # [N, D]
    norm_d = np.maximum(np.sqrt(np.sum(tf64 * tf64, axis=0)), EPS)
    q64 = test_features.astype(np.float64)
    qn = np.sqrt(np.sum(q64 * q64, axis=1, keepdims=True))
    q_scaled = q64 / np.maximum(qn, EPS) / norm_d        # [B, D] f64

    # per-query fp8 scale so entries have rms ~8 (well inside e4m3 range)
    sigma_b = np.sqrt(np.sum(q_scaled * q_scaled, axis=1))   # exact sim std
    s_b = 128.0 / sigma_b                                 # [B]
    q8 = (q_scaled * s_b[:, None]).astype(FP8)            # [B, D]
    g8 = train_features.T.astype(FP8)                     # [D, N]

    # device-side sim std (from the actual quantized values)
    q8f = q8.astype(np.float64)
    g8f64_sq_mean = float(np.mean(g8.astype(np.float32) ** 2))
    sig_dev = np.sqrt(np.sum(q8f * q8f, axis=1) * g8f64_sq_mean)  # [B]
    tau_dev = TAU_Z * sig_dev                             # [B]

    # ---- pack device inputs ----
    q_packed = np.ascontiguousarray(
        q8.T.reshape(2, 128, B).transpose(1, 0, 2))       # [128, 2, B]
    g_shards = []
    for core in range(NCORES):
        sl = np.zeros((2, 128, NPC_PAD), dtype=FP8)
        sl[:, :, :NPC] = g8[:, core * NPC:(core + 1) * NPC].reshape(2, 128, NPC)
        g_shards.append(np.ascontiguousarray(sl.transpose(1, 0, 2)))
    tau_packed = np.ascontiguousarray(
        (-tau_dev).astype(np.float32).reshape(NCH, 128).T)    # [128, NCH]

    # ---- device: fp8 matmul + 2-engine screen ----
    if os.environ.get("KNN_EMULATE"):
        dves, acts = _run_emulated(g_shards, q_packed, tau_packed)
    else:
        dves, acts = _run_device(g_shards, q_packed, tau_packed)

    # ---- host: flag blocks, exact f64 rescore, top-10, softmax ----
    # universe of 512-blocks: 8 cores x 50; block (core, k) covers global
    # cols [core*NPC + 512k, core*NPC + min(512k+512, NPC))
    NBLK = NPC_PAD // 512                                 # 50 per core
    flags = np.zeros((B, NCORES, NBLK), dtype=bool)
    for core in range(NCORES):
        dve = dves[core].astype(np.float64)               # [NCH,128,26]
        act = acts[core].astype(np.float64)               # [NCH,128,13]
        for c in range(NCH):
            brow = slice(c * 128, (c + 1) * 128)
            tt = tau_dev[brow]                            # [128]
            for j in range(NGR):
                o = j // 2
                if (j + c) % 2 == 0:
                    m = dve[c, :, 2 * o:2 * o + 2]        # [128, 2]
                    f = m >= tt[:, None]
                    flags[brow, core, 2 * j:2 * j + 2] |= f
                else:
                    f = act[c, :, o] > 0.0
                    flags[brow, core, 2 * j] |= f
                    flags[brow, core, 2 * j + 1] |= f

    flags = flags.reshape(B, NCORES * NBLK)
    # per-segment query lists
    seg_queries = [np.nonzero(flags[:, s])[0] for s in range(NCORES * NBLK)]

    per_q_vals = [[] for _ in range(B)]
    per_q_cols = [[] for _ in range(B)]
    for s, qs in enumerate(seg_queries):
        if len(qs) == 0:
            continue
        core, k = divmod(s, NBLK)
        c0 = core * NPC + 512 * k
        c1 = core * NPC + min(512 * k + 512, NPC)
        if c0 >= c1:
            continue
        block = tf64[c0:c1]                               # [w, D]
        sims = q_scaled[qs] @ block.T                     # [nq, w] f64
        cols = np.arange(c0, c1)
        for i, b in enumerate(qs):
            per_q_vals[b].append(sims[i])
            per_q_cols[b].append(cols)

    scores = np.zeros((B, NUM_CLASSES), dtype=np.float64)
    fallback = []
    for b in range(B):
        if per_q_vals[b]:
            v = np.concatenate(per_q_vals[b])
            cidx = np.concatenate(per_q_cols[b])
        else:
            v = np.empty(0)
            cidx = np.empty(0, np.int64)
        if len(v) < NB_KNN:
            fallback.append(b)
            continue
        sel = np.argpartition(-v, NB_KNN - 1)[:NB_KNN]
        # certificate: 10th best must clear tau + noise margin (device units)
        v10_dev = s_b[b] * np.sort(v[sel])[0]
        if v10_dev <= tau_dev[b] + CERT_Z * sig_dev[b]:
            fallback.append(b)
            continue
        order = np.lexsort((cidx[sel], -v[sel]))
        sel = sel[order]
        topv = v[sel]
        w = np.exp(topv / T - np.max(topv) / T)
        w /= w.sum()
        np.add.at(scores[b], labels[cidx[sel]], w)

    if fallback:
        fb = np.asarray(fallback)
        sims = q_scaled[fb] @ tf64.T                      # [nfb, N] f64
        for i, b in enumerate(fb):
            v = sims[i]
            sel = np.argpartition(-v, NB_KNN - 1)[:NB_KNN]
            order = np.lexsort((sel, -v[sel]))
            sel = sel[order]
            topv = v[sel]
            w = np.exp(topv / T - np.max(topv) / T)
            w /= w.sum()
            np.add.at(scores[b], labels[sel], w)

    return scores.astype(np.float32)


if __name__ == "__main__":
    rng = np.random.default_rng(0)
    tf = rng.standard_normal((B, D), dtype=np.float32)
    trf = rng.standard_normal((N, D), dtype=np.float32)
    trl = rng.integers(0, NUM_CLASSES, N).astype(np.int64)
    os.environ["KNN_EMULATE"] = "1"
    out = kernel(tf, trf, trl)
    print(out.shape, out.dtype, out.sum())


# revision 4
# speedup vs baseline: 1.8586x; 1.0290x over previous
"""KNN classification kernel for Trainium2 (8 NeuronCores), v2.

Problem: B=1024 queries x N=200000 gallery, D=256, top-10 neighbors,
softmax-weighted one-hot class scores over 50 classes.

Math fold: reference computes gallery = l2norm(train.T, axis=1) -- each
feature dim d is scaled by 1/||train[:, d]|| over the FULL gallery. That
folds into the query side, so the device only needs q_scaled @ train.T.

Device (per core, gallery sharded along N into 8 x 25000, zero-padded to
25088 = 24.5 granules x 1024):
  PE:  fp8e4 DoubleRow matmuls (K=256 packed as [128,2]) -> sim granule
       [128q, 1024] f32 in PSUM (0.5 cycles/row = 4x bf16 throughput)
  Screen (the bottleneck) split across two engines, alternating granules:
   - DVE tensor_reduce(max, axis=X) over [128,4,256] -> four per-256 maxes
   - ACT Relu(x - tau_b) with accum_out -> per-1024 exceedance sum, where
     tau_b = 3.25 * sigma_dev(b) is a per-query threshold (bias AP)
  PSUM ring: 4 granule buffers x 2 banks. Loop is gallery-block-major so
  each gallery DMA block is consumed by all 8 query chunks before the next
  block is needed (hides all gallery DMA after the first block).
Host: flag 256-blocks (DVE max >= tau) and 1024-granules (relu sum > 0),
  rescore flagged columns exactly in f64, exact top-10 -> softmax scores.
  Certificate: found 10th value must clear tau + 5.7 sigma_noise, else that
  query falls back to a full exact rescore (probability ~1e-7).
Safety: any exact-top-10 item has z >= ~3.8 sigma whp while tau = 3.25
  sigma; fp8 dot-product noise is ~0.06 sigma, so a top-10 item landing
  under the device-side threshold needs a ~10-sigma-noise deviation.
"""

import os
import numpy as np

NB_KNN = 10
T = 0.07
NUM_CLASSES = 50
EPS = 1e-12

B, N, D = 1024, 200000, 256
NCORES = 8
NPC = N // NCORES           # 25000 real cols per core
GR = 1024                   # granule width (2 PSUM banks)
NGR = 25                    # granules per chunk (last one is half width)
NPC_PAD = 25088             # 24 x 1024 + 512
NCH = 8                     # query chunks of 128
# gallery DMA blocks, in granules (last granule is 512 cols)
BLK_GR = [1, 4, 5, 5, 5, 4, 1]
SUB = 256                   # DVE max sub-block width
TAU_Z = 3.25                # screen threshold in device-sigma units
CERT_Z = 0.33               # certificate margin in device-sigma units

_CACHE = {}


def _gr_width(j):
    return 512 if j == NGR - 1 else GR


def _gr_col(j):
    return j * GR


def _build_bass(double_row=True):
    import concourse.bacc as bacc
    import concourse.tile as tile
    from concourse import mybir

    nc = bacc.Bacc("TRN2")
    f32 = mybir.dt.float32
    fp8 = mybir.dt.float8e4

    g_d = nc.dram_tensor("g", [128, 2, NPC_PAD], fp8, kind="ExternalInput")
    q_d = nc.dram_tensor("q", [128, 2, B], fp8, kind="ExternalInput")
    tau_d = nc.dram_tensor("tau", [128, NCH], f32, kind="ExternalInput")
    # per chunk: 13*4 per-256 maxes (DVE), 13 relu sums (ACT); half granule
    # 24 contributes 2 maxes or 1 sum depending on parity
    dve_d = nc.dram_tensor("dve", [NCH, 128, 52], f32, kind="ExternalOutput")
    act_d = nc.dram_tensor("act", [NCH, 128, 13], f32, kind="ExternalOutput")

    X = mybir.AxisListType.X
    MAX = mybir.AluOpType.max
    RELU = mybir.ActivationFunctionType.Relu
    pm = mybir.MatmulPerfMode.DoubleRow if double_row else None

    with tile.TileContext(nc) as tc:
        with tc.tile_pool(name="qp", bufs=1) as qp, \
             tc.tile_pool(name="gp", bufs=1) as gp, \
             tc.tile_pool(name="op", bufs=1) as op, \
             tc.tile_pool(name="pp", bufs=1, space="PSUM") as pp:
            q8 = qp.tile([128, 2, B], fp8, tag="q8")
            tau = qp.tile([128, NCH], f32, tag="tau")
            nc.sync.dma_start(out=q8[:], in_=q_d[:])
            nc.sync.dma_start(out=tau[:], in_=tau_d[:])

            gal = []
            g0 = 0
            for i, ng in enumerate(BLK_GR):
                c0 = _gr_col(g0)
                cw = sum(_gr_width(g0 + k) for k in range(ng))
                t = gp.tile([128, 2, cw], fp8, tag=f"gal{i}")
                nc.sync.dma_start(out=t[:], in_=g_d[:, :, c0:c0 + cw])
                gal.append((t, g0, c0))
                g0 += ng

            dve_o = [op.tile([128, 52], f32, tag=f"dve{c}",
                             name=f"dve_o{c}") for c in range(NCH)]
            act_o = [op.tile([128, 13], f32, tag=f"act{c}",
                             name=f"act_o{c}") for c in range(NCH)]

            for i, ng in enumerate(BLK_GR):
                t, gbase, cbase = gal[i]
                for c in range(NCH):
                    lhs = q8[:, :, c * 128:(c + 1) * 128]
                    for k in range(ng):
                        j = gbase + k
                        w = _gr_width(j)
                        l0 = _gr_col(j) - cbase
                        ps = pp.tile([128, GR], f32, tag="ps", bufs=4)
                        if double_row:
                            nc.tensor.matmul(ps[:, :w], lhs,
                                             t[:, :, l0:l0 + w],
                                             start=True, stop=True,
                                             perf_mode=pm)
                        else:
                            nc.tensor.matmul(ps[:, :w], lhs[:, 0],
                                             t[:, 0, l0:l0 + w],
                                             start=True, stop=False)
                            nc.tensor.matmul(ps[:, :w], lhs[:, 1],
                                             t[:, 1, l0:l0 + w],
                                             start=False, stop=True)
                        o = j // 2
                        nsub = w // SUB
                        if (j + c) % 2 == 0:
                            nc.vector.tensor_reduce(
                                dve_o[c][:, 4 * o:4 * o + nsub],
                                ps[:, :w].rearrange("p (r w) -> p r w",
                                                    r=nsub),
                                axis=X, op=MAX)
                        else:
                            nc.scalar.activation(
                                out=ps[:, :w], in_=ps[:, :w], func=RELU,
                                bias=tau[:, c:c + 1], scale=1.0,
                                accum_out=act_o[c][:, o:o + 1])
                    if i == len(BLK_GR) - 1:
                        nc.sync.dma_start(out=dve_d[c], in_=dve_o[c][:])
                        nc.sync.dma_start(out=act_d[c], in_=act_o[c][:])
    if not nc.is_finalized():
        nc.finalize()
    return nc


def _run_device(g_shards, q_packed, tau_packed):
    from concourse.bass_utils import run_bass_kernel_spmd
    if "nc" not in _CACHE:
        _CACHE["nc"] = _build_bass()
    nc = _CACHE["nc"]
    in_maps = [{"g": g_shards[c], "q": q_packed, "tau": tau_packed}
               for c in range(NCORES)]
    res = run_bass_kernel_spmd(nc, in_maps, list(range(NCORES)))
    return ([res.results[c]["dve"] for c in range(NCORES)],
            [res.results[c]["act"] for c in range(NCORES)])


def _run_emulated(g_shards, q_packed, tau_packed):
    """Numpy emulation of the device kernel (same outputs)."""
    qf = q_packed.astype(np.float32)         # [128, 2, B]
    dves, acts = [], []
    for core in range(NCORES):
        gf = g_shards[core].astype(np.float32)   # [128, 2, NPC_PAD]
        sim = np.einsum("pib,pin->bn", qf, gf)   # [B, NPC_PAD]
        dve = np.zeros((NCH, 128, 52), np.float32)
        act = np.zeros((NCH, 128, 13), np.float32)
        for c in range(NCH):
            sc = sim[c * 128:(c + 1) * 128]      # [128, NPC_PAD]
            for j in range(NGR):
                o = j // 2
                w = _gr_width(j)
                gsl = sc[:, _gr_col(j):_gr_col(j) + w]
                if (j + c) % 2 == 0:
                    nsub = w // SUB
                    m = gsl.reshape(128, nsub, SUB).max(axis=2)
                    dve[c, :, 4 * o:4 * o + nsub] = m
                else:
                    bias = tau_packed[:, c:c + 1]
                    act[c, :, o] = np.maximum(gsl + bias, 0).sum(axis=1)
        dves.append(dve)
        acts.append(act)
    return dves, acts


def kernel(test_features, train_features, train_labels):
    import ml_dtypes
    FP8 = ml_dtypes.float8_e4m3fn

    test_features = np.asarray(test_features, dtype=np.float32)
    train_features = np.asarray(train_features, dtype=np.float32)
    labels = np.asarray(train_labels).astype(np.int64)

    # ---- host pre: fold normalizations into the query side ----
    tf64 = train_features.astype(np.float64)             # [N, D]
    norm_d = np.maximum(np.sqrt(np.sum(tf64 * tf64, axis=0)), EPS)
    q64 = test_features.astype(np.float64)
    qn = np.sqrt(np.sum(q64 * q64, axis=1, keepdims=True))
    q_scaled = q64 / np.maximum(qn, EPS) / norm_d        # [B, D] f64

    # per-query fp8 scale so entries have rms ~8 (well inside e4m3 range)
    sigma_b = np.sqrt(np.sum(q_scaled * q_scaled, axis=1))   # exact sim std
    s_b = 128.0 / sigma_b                                 # [B]
    q8 = (q_scaled * s_b[:, None]).astype(FP8)            # [B, D]
    g8 = train_features.T.astype(FP8)                     # [D, N]

    # device-side sim std (from the actual quantized values)
    q8f = q8.astype(np.float64)
    g8_sq_mean = float(np.mean(g8.astype(np.float32) ** 2))
    sig_dev = np.sqrt(np.sum(q8f * q8f, axis=1) * g8_sq_mean)  # [B]
    tau_dev = TAU_Z * sig_dev                             # [B]

    # ---- pack device inputs ----
    q_packed = np.ascontiguousarray(
        q8.T.reshape(2, 128, B).transpose(1, 0, 2))       # [128, 2, B]
    g_shards = []
    for core in range(NCORES):
        sl = np.zeros((2, 128, NPC_PAD), dtype=FP8)
        sl[:, :, :NPC] = g8[:, core * NPC:(core + 1) * NPC].reshape(2, 128, NPC)
        g_shards.append(np.ascontiguousarray(sl.transpose(1, 0, 2)))
    tau_packed = np.ascontiguousarray(
        (-tau_dev).astype(np.float32).reshape(NCH, 128).T)    # [128, NCH]

    # ---- device: fp8 matmul + 2-engine screen ----
    if os.environ.get("KNN_EMULATE"):
        dves, acts = _run_emulated(g_shards, q_packed, tau_packed)
    else:
        dves, acts = _run_device(g_shards, q_packed, tau_packed)

    # ---- host: flag 256-blocks, exact f64 rescore, top-10, softmax ----
    NBLK = NPC_PAD // SUB                                 # 98 per core
    flags = np.zeros((B, NCORES, NBLK), dtype=bool)
    for core in range(NCORES):
        dve = dves[core].astype(np.float64)               # [NCH,128,52]
        act = acts[core].astype(np.float64)               # [NCH,128,13]
        for c in range(NCH):
            brow = slice(c * 128, (c + 1) * 128)
            tt = tau_dev[brow]                            # [128]
            for j in range(NGR):
                o = j // 2
                w = _gr_width(j)
                nsub = w // SUB
                k0 = _gr_col(j) // SUB
                if (j + c) % 2 == 0:
                    m = dve[c, :, 4 * o:4 * o + nsub]     # [128, nsub]
                    flags[brow, core, k0:k0 + nsub] |= m >= tt[:, None]
                else:
                    f = act[c, :, o] > 0.0
                    flags[brow, core, k0:k0 + nsub] |= f[:, None]

    flags = flags.reshape(B, NCORES * NBLK)
    seg_queries = [np.nonzero(flags[:, s])[0] for s in range(NCORES * NBLK)]

    per_q_vals = [[] for _ in range(B)]
    per_q_cols = [[] for _ in range(B)]
    for s, qs in enumerate(seg_queries):
        if len(qs) == 0:
            continue
        core, k = divmod(s, NBLK)
        c0 = core * NPC + SUB * k
        c1 = core * NPC + min(SUB * k + SUB, NPC)
        if c0 >= c1:
            continue
        block = tf64[c0:c1]                               # [w, D]
        sims = q_scaled[qs] @ block.T                     # [nq, w] f64
        cols = np.arange(c0, c1)
        for i, b in enumerate(qs):
            per_q_vals[b].append(sims[i])
            per_q_cols[b].append(cols)

    scores = np.zeros((B, NUM_CLASSES), dtype=np.float64)
    fallback = []
    for b in range(B):
        if per_q_vals[b]:
            v = np.concatenate(per_q_vals[b])
            cidx = np.concatenate(per_q_cols[b])
        else:
            v = np.empty(0)
            cidx = np.empty(0, np.int64)
        if len(v) < NB_KNN:
            fallback.append(b)
            continue
        sel = np.argpartition(-v, NB_KNN - 1)[:NB_KNN]
        # certificate: 10th best must clear tau + noise margin (device units)
        v10_dev = s_b[b] * np.sort(v[sel])[0]
        if v10_dev <= tau_dev[b] + CERT_Z * sig_dev[b]:
            fallback.append(b)
            continue
        order = np.lexsort((cidx[sel], -v[sel]))
        sel = sel[order]
        topv = v[sel]
        w = np.exp(topv / T - np.max(topv) / T)
        w /= w.sum()
        np.add.at(scores[b], labels[cidx[sel]], w)

    if fallback:
        fb = np.asarray(fallback)
        sims = q_scaled[fb] @ tf64.T                      # [nfb, N] f64
        for i, b in enumerate(fb):
            v = sims[i]
            sel = np.argpartition(-v, NB_KNN - 1)[:NB_KNN]
            order = np.lexsort((sel, -v[sel]))
            sel = sel[order]
            topv = v[sel]
            w = np.exp(topv / T - np.max(topv) / T)
            w /= w.sum()
            np.add.at(scores[b], labels[sel], w)

    return scores.astype(np.float32)


if __name__ == "__main__":
    rng = np.random.default_rng(0)
    tf = rng.standard_normal((B, D), dtype=np.float32)
    trf = rng.standard_normal((N, D), dtype=np.float32)
    trl = rng.integers(0, NUM_CLASSES, N).astype(np.int64)
    os.environ["KNN_EMULATE"] = "1"
    out = kernel(tf, trf, trl)
    print(out.shape, out.dtype, out.sum())


# revision 5
# speedup vs baseline: 1.8848x; 1.0141x over previous
"""KNN classification kernel for Trainium2 (8 NeuronCores), v2.

Problem: B=1024 queries x N=200000 gallery, D=256, top-10 neighbors,
softmax-weighted one-hot class scores over 50 classes.

Math fold: reference computes gallery = l2norm(train.T, axis=1) -- each
feature dim d is scaled by 1/||train[:, d]|| over the FULL gallery. That
folds into the query side, so the device only needs q_scaled @ train.T.

Device (per core, gallery sharded along N into 8 x 25000, zero-padded to
25088 = 24.5 granules x 1024):
  PE:  fp8e4 DoubleRow matmuls (K=256 packed as [128,2]) -> sim granule
       [128q, 1024] f32 in PSUM (0.5 cycles/row = 4x bf16 throughput)
  Screen (the bottleneck) split across two engines, alternating granules:
   - DVE tensor_reduce(max, axis=X) over [128,4,256] -> four per-256 maxes
   - ACT Relu(x - tau_b) with accum_out -> per-1024 exceedance sum, where
     tau_b = 3.25 * sigma_dev(b) is a per-query threshold (bias AP)
  PSUM ring: 4 granule buffers x 2 banks. Loop is gallery-block-major so
  each gallery DMA block is consumed by all 8 query chunks before the next
  block is needed (hides all gallery DMA after the first block).
Host: flag 256-blocks (DVE max >= tau) and 1024-granules (relu sum > 0),
  rescore flagged columns exactly in f64, exact top-10 -> softmax scores.
  Certificate: found 10th value must clear tau + 5.7 sigma_noise, else that
  query falls back to a full exact rescore (probability ~1e-7).
Safety: any exact-top-10 item has z >= ~3.8 sigma whp while tau = 3.25
  sigma; fp8 dot-product noise is ~0.06 sigma, so a top-10 item landing
  under the device-side threshold needs a ~10-sigma-noise deviation.
"""

import os
import numpy as np

NB_KNN = 10
T = 0.07
NUM_CLASSES = 50
EPS = 1e-12

B, N, D = 1024, 200000, 256
NCORES = 8
NPC = N // NCORES           # 25000 real cols per core
GR = 1024                   # granule width (2 PSUM banks)
NGR = 25                    # granules per chunk (last one is half width)
NPC_PAD = 25088             # 24 x 1024 + 512
NCH = 8                     # query chunks of 128
# gallery DMA blocks, in granules (last granule is 512 cols)
BLK_GR = [1, 4, 5, 5, 5, 4, 1]
SUB = 256                   # DVE max sub-block width
TAU_Z = 3.25                # screen threshold in device-sigma units
CERT_Z = 0.33               # certificate margin in device-sigma units

_CACHE = {}


def _gr_width(j):
    return 512 if j == NGR - 1 else GR


def _gr_col(j):
    return j * GR


def _build_bass(double_row=True):
    import concourse.bacc as bacc
    import concourse.tile as tile
    from concourse import mybir

    nc = bacc.Bacc("TRN2")
    f32 = mybir.dt.float32
    fp8 = mybir.dt.float8e4

    g_d = nc.dram_tensor("g", [128, 2, NPC_PAD], fp8, kind="ExternalInput")
    q_d = nc.dram_tensor("q", [128, 2, B], fp8, kind="ExternalInput")
    tau_d = nc.dram_tensor("tau", [128, NCH], f32, kind="ExternalInput")
    # per chunk: 13*4 per-256 maxes (DVE), 13 relu sums (ACT); half granule
    # 24 contributes 2 maxes or 1 sum depending on parity
    dve_d = nc.dram_tensor("dve", [128, NCH * 52], f32, kind="ExternalOutput")
    act_d = nc.dram_tensor("act", [128, NCH * 13], f32, kind="ExternalOutput")

    X = mybir.AxisListType.X
    MAX = mybir.AluOpType.max
    RELU = mybir.ActivationFunctionType.Relu
    pm = mybir.MatmulPerfMode.DoubleRow if double_row else None

    with tile.TileContext(nc) as tc:
        with tc.tile_pool(name="qp", bufs=1) as qp, \
             tc.tile_pool(name="gp", bufs=1) as gp, \
             tc.tile_pool(name="op", bufs=1) as op, \
             tc.tile_pool(name="pp", bufs=1, space="PSUM") as pp:
            q8 = qp.tile([128, 2, B], fp8, tag="q8")
            tau = qp.tile([128, NCH], f32, tag="tau")
            gal = []
            g0 = 0
            gtiles = []
            for i, ng in enumerate(BLK_GR):
                c0 = _gr_col(g0)
                cw = sum(_gr_width(g0 + k) for k in range(ng))
                t = gp.tile([128, 2, cw], fp8, tag=f"gal{i}", name=f"gal_t{i}")
                gal.append((t, g0, c0))
                gtiles.append((t, c0, cw))
                g0 += ng
            # DMA order tuned for pipeline head: first gallery block, the
            # first query chunk, then the rest
            t0, c00, cw0 = gtiles[0]
            nc.sync.dma_start(out=t0[:], in_=g_d[:, :, c00:c00 + cw0])
            nc.sync.dma_start(out=q8[:, :, 0:128], in_=q_d[:, :, 0:128])
            nc.sync.dma_start(out=tau[:], in_=tau_d[:])
            nc.sync.dma_start(out=q8[:, :, 128:B], in_=q_d[:, :, 128:B])
            for t, c0, cw in gtiles[1:]:
                nc.sync.dma_start(out=t[:], in_=g_d[:, :, c0:c0 + cw])

            dve_o = op.tile([128, NCH * 52], f32, tag="dve_o")
            act_o = op.tile([128, NCH * 13], f32, tag="act_o")

            for i, ng in enumerate(BLK_GR):
                t, gbase, cbase = gal[i]
                for c in range(NCH):
                    lhs = q8[:, :, c * 128:(c + 1) * 128]
                    for k in range(ng):
                        j = gbase + k
                        w = _gr_width(j)
                        l0 = _gr_col(j) - cbase
                        ps = pp.tile([128, GR], f32, tag="ps", bufs=4)
                        if double_row:
                            nc.tensor.matmul(ps[:, :w], lhs,
                                             t[:, :, l0:l0 + w],
                                             start=True, stop=True,
                                             perf_mode=pm)
                        else:
                            nc.tensor.matmul(ps[:, :w], lhs[:, 0],
                                             t[:, 0, l0:l0 + w],
                                             start=True, stop=False)
                            nc.tensor.matmul(ps[:, :w], lhs[:, 1],
                                             t[:, 1, l0:l0 + w],
                                             start=False, stop=True)
                        o = j // 2
                        nsub = w // SUB
                        if (j + c) % 2 == 0:
                            d0 = c * 52 + 4 * o
                            nc.vector.tensor_reduce(
                                dve_o[:, d0:d0 + nsub],
                                ps[:, :w].rearrange("p (r w) -> p r w",
                                                    r=nsub),
                                axis=X, op=MAX)
                        else:
                            a0 = c * 13 + o
                            nc.scalar.activation(
                                out=ps[:, :w], in_=ps[:, :w], func=RELU,
                                bias=tau[:, c:c + 1], scale=1.0,
                                accum_out=act_o[:, a0:a0 + 1])
            nc.sync.dma_start(out=dve_d[:], in_=dve_o[:])
            nc.sync.dma_start(out=act_d[:], in_=act_o[:])
    if not nc.is_finalized():
        nc.finalize()
    return nc


def _run_device(g_shards, q_packed, tau_packed):
    from concourse.bass_utils import run_bass_kernel_spmd
    if "nc" not in _CACHE:
        _CACHE["nc"] = _build_bass()
    nc = _CACHE["nc"]
    in_maps = [{"g": g_shards[c], "q": q_packed, "tau": tau_packed}
               for c in range(NCORES)]
    res = run_bass_kernel_spmd(nc, in_maps, list(range(NCORES)))
    return ([res.results[c]["dve"] for c in range(NCORES)],
            [res.results[c]["act"] for c in range(NCORES)])


def _run_emulated(g_shards, q_packed, tau_packed):
    """Numpy emulation of the device kernel (same outputs)."""
    qf = q_packed.astype(np.float32)         # [128, 2, B]
    dves, acts = [], []
    for core in range(NCORES):
        gf = g_shards[core].astype(np.float32)   # [128, 2, NPC_PAD]
        sim = np.einsum("pib,pin->bn", qf, gf)   # [B, NPC_PAD]
        dve = np.zeros((NCH, 128, 52), np.float32)
        act = np.zeros((NCH, 128, 13), np.float32)
        for c in range(NCH):
            sc = sim[c * 128:(c + 1) * 128]      # [128, NPC_PAD]
            for j in range(NGR):
                o = j // 2
                w = _gr_width(j)
                gsl = sc[:, _gr_col(j):_gr_col(j) + w]
                if (j + c) % 2 == 0:
                    nsub = w // SUB
                    m = gsl.reshape(128, nsub, SUB).max(axis=2)
                    dve[c, :, 4 * o:4 * o + nsub] = m
                else:
                    bias = tau_packed[:, c:c + 1]
                    act[c, :, o] = np.maximum(gsl + bias, 0).sum(axis=1)
        dves.append(dve)
        acts.append(act)
    return dves, acts


def kernel(test_features, train_features, train_labels):
    import ml_dtypes
    FP8 = ml_dtypes.float8_e4m3fn

    test_features = np.asarray(test_features, dtype=np.float32)
    train_features = np.asarray(train_features, dtype=np.float32)
    labels = np.asarray(train_labels).astype(np.int64)

    # ---- host pre: fold normalizations into the query side ----
    tf64 = train_features.astype(np.float64)             # [N, D]
    norm_d = np.maximum(np.sqrt(np.sum(tf64 * tf64, axis=0)), EPS)
    q64 = test_features.astype(np.float64)
    qn = np.sqrt(np.sum(q64 * q64, axis=1, keepdims=True))
    q_scaled = q64 / np.maximum(qn, EPS) / norm_d        # [B, D] f64

    # per-query fp8 scale so entries have rms ~8 (well inside e4m3 range)
    sigma_b = np.sqrt(np.sum(q_scaled * q_scaled, axis=1))   # exact sim std
    s_b = 128.0 / sigma_b                                 # [B]
    q8 = (q_scaled * s_b[:, None]).astype(FP8)            # [B, D]
    g8 = train_features.T.astype(FP8)                     # [D, N]

    # device-side sim std (from the actual quantized values)
    q8f = q8.astype(np.float64)
    g8_sq_mean = float(np.mean(g8.astype(np.float32) ** 2))
    sig_dev = np.sqrt(np.sum(q8f * q8f, axis=1) * g8_sq_mean)  # [B]
    tau_dev = TAU_Z * sig_dev                             # [B]

    # ---- pack device inputs ----
    q_packed = np.ascontiguousarray(
        q8.T.reshape(2, 128, B).transpose(1, 0, 2))       # [128, 2, B]
    g_shards = []
    for core in range(NCORES):
        sl = np.zeros((2, 128, NPC_PAD), dtype=FP8)
        sl[:, :, :NPC] = g8[:, core * NPC:(core + 1) * NPC].reshape(2, 128, NPC)
        g_shards.append(np.ascontiguousarray(sl.transpose(1, 0, 2)))
    tau_packed = np.ascontiguousarray(
        (-tau_dev).astype(np.float32).reshape(NCH, 128).T)    # [128, NCH]

    # ---- device: fp8 matmul + 2-engine screen ----
    if os.environ.get("KNN_EMULATE"):
        dves, acts = _run_emulated(g_shards, q_packed, tau_packed)
    else:
        dves, acts = _run_device(g_shards, q_packed, tau_packed)

    # ---- host: flag 256-blocks, exact f64 rescore, top-10, softmax ----
    NBLK = NPC_PAD // SUB                                 # 98 per core
    flags = np.zeros((B, NCORES, NBLK), dtype=bool)
    for core in range(NCORES):
        dve = dves[core].astype(np.float64)
        act = acts[core].astype(np.float64)
        if dve.ndim == 2:    # [128, NCH*52] device layout -> [NCH,128,52]
            dve = dve.reshape(128, NCH, 52).transpose(1, 0, 2)
            act = act.reshape(128, NCH, 13).transpose(1, 0, 2)
        for c in range(NCH):
            brow = slice(c * 128, (c + 1) * 128)
            tt = tau_dev[brow]                            # [128]
            for j in range(NGR):
                o = j // 2
                w = _gr_width(j)
                nsub = w // SUB
                k0 = _gr_col(j) // SUB
                if (j + c) % 2 == 0:
                    m = dve[c, :, 4 * o:4 * o + nsub]     # [128, nsub]
                    flags[brow, core, k0:k0 + nsub] |= m >= tt[:, None]
                else:
                    f = act[c, :, o] > 0.0
                    flags[brow, core, k0:k0 + nsub] |= f[:, None]

    flags = flags.reshape(B, NCORES * NBLK)
    seg_queries = [np.nonzero(flags[:, s])[0] for s in range(NCORES * NBLK)]

    per_q_vals = [[] for _ in range(B)]
    per_q_cols = [[] for _ in range(B)]
    for s, qs in enumerate(seg_queries):
        if len(qs) == 0:
            continue
        core, k = divmod(s, NBLK)
        c0 = core * NPC + SUB * k
        c1 = core * NPC + min(SUB * k + SUB, NPC)
        if c0 >= c1:
            continue
        block = tf64[c0:c1]                               # [w, D]
        sims = q_scaled[qs] @ block.T                     # [nq, w] f64
        cols = np.arange(c0, c1)
        for i, b in enumerate(qs):
            per_q_vals[b].append(sims[i])
            per_q_cols[b].append(cols)

    scores = np.zeros((B, NUM_CLASSES), dtype=np.float64)
    fallback = []
    for b in range(B):
        if per_q_vals[b]:
            v = np.concatenate(per_q_vals[b])
            cidx = np.concatenate(per_q_cols[b])
        else:
            v = np.empty(0)
            cidx = np.empty(0, np.int64)
        if len(v) < NB_KNN:
            fallback.append(b)
            continue
        sel = np.argpartition(-v, NB_KNN - 1)[:NB_KNN]
        # certificate: 10th best must clear tau + noise margin (device units)
        v10_dev = s_b[b] * np.sort(v[sel])[0]
        if v10_dev <= tau_dev[b] + CERT_Z * sig_dev[b]:
            fallback.append(b)
            continue
        order = np.lexsort((cidx[sel], -v[sel]))
        sel = sel[order]
        topv = v[sel]
        w = np.exp(topv / T - np.max(topv) / T)
        w /= w.sum()
        np.add.at(scores[b], labels[cidx[sel]], w)

    if fallback:
        fb = np.asarray(fallback)
        sims = q_scaled[fb] @ tf64.T                      # [nfb, N] f64
        for i, b in enumerate(fb):
            v = sims[i]
            sel = np.argpartition(-v, NB_KNN - 1)[:NB_KNN]
            order = np.lexsort((sel, -v[sel]))
            sel = sel[order]
            topv = v[sel]
            w = np.exp(topv / T - np.max(topv) / T)
            w /= w.sum()
            np.add.at(scores[b], labels[sel], w)

    return scores.astype(np.float32)


if __name__ == "__main__":
    rng = np.random.default_rng(0)
    tf = rng.standard_normal((B, D), dtype=np.float32)
    trf = rng.standard_normal((N, D), dtype=np.float32)
    trl = rng.integers(0, NUM_CLASSES, N).astype(np.int64)
    os.environ["KNN_EMULATE"] = "1"
    out = kernel(tf, trf, trl)
    print(out.shape, out.dtype, out.sum())
